# revision 24
# baseline (speedup 1.0000x reference)
"""CapsuleNet dynamic-routing Trainium2 kernel (nn_Capsule_54657753809237).

kernel(u, W) takes the FULL inputs (u [256,1152,8] f32, W [1152,8,160] f32)
and returns the FULL output v [256,10,16] f32.

Strategy: data-parallel over batch (32 samples/core, 8 cores), W replicated.
Per core, samples are processed in 4 chunks of 8. Partition layout packs
(b=8, nn=16) into the 128 partitions with n = g*16+nn (g=72 groups):

- uhat = einsum('bni,nim->bnm') is computed per g as ONE PE matmul with a
  block-diagonal stationary built from u ([nn*8+i, b*16+nn'] nonzero iff
  nn==nn') streaming W's g-slice [128,160] -> uhat[(b,nn), (o,d)] lands
  directly in the routing layout. uhat never touches HBM (189 MB saved).
- Routing iteration weighted-sum s = sum_n c*uhat runs on the PE: stationary
  block-diagonal c [(b,nn),(b',o)] (nonzero iff b==b'), rhs = uhat g-slice,
  PSUM-accumulated over all 72 g -> [80,160]; the (o==o') diagonal is
  extracted with a constant mask + tiny reduce. Iteration 0 uses a constant
  ones block-diagonal (softmax of zeros is uniform) scaled by 0.1 afterwards.
- Agreement a = sum_d uhat*v runs on DVE/GPSIMD (elementwise + X-reduce),
  with v broadcast across partitions via a tiny PE matmul.
- softmax over o (free dim) and squash are small DVE/ACT ops.
"""

import numpy as np
import ml_dtypes

B, NIN, IDIM, NOUT, ODIM = 256, 1152, 8, 10, 16
OD = NOUT * ODIM            # 160
NCORES = 8
BCORE = B // NCORES         # 32 samples per core
BC = 8                      # samples per chunk
NCHUNK = BCORE // BC        # 4
NN = 16                     # capsules per group
G = NIN // NN               # 72 groups
P = 128                     # partitions = BC*NN
Q = BC * NOUT               # 80 = R1 output partitions
GSUB = 18                   # g's per R2 block (4 blocks)

_STATE = {}


def _split_waits_json(bir):
    """walrus in this container rejects >1 sync-wait per instruction ("Too
    many sync wait commands"). Rewrite the BIR: move extra waits onto NoOps
    injected just before the instruction on the same engine - the engine
    blocks on the NoOps first, so semantics are unchanged."""
    import json
    m = json.loads(bir)
    counter = [0]

    def fix_block(bb):
        insts = bb.get("instructions")
        if not insts:
            return
        out = []
        changed = False
        for inst in insts:
            si = inst.get("sync_info")
            waits = (si or {}).get("on_wait") or []
            if len(waits) > 1:
                changed = True
                for w in waits[:-1]:
                    counter[0] += 1
                    out.append({
                        "debug": inst.get("debug", 0),
                        "engine": inst["engine"],
                        "ins": [],
                        "outs": [],
                        "name": f"I-wsplit{counter[0]}",
                        "opcode": "NoOp",
                        "sync_info": {"on_wait": [w], "on_update": []},
                    })
                si["on_wait"] = waits[-1:]
            out.append(inst)
        if changed:
            bb["instructions"] = out

    def walk(o):
        if isinstance(o, dict):
            if "instructions" in o:
                fix_block(o)
            for v in o.values():
                walk(v)
        elif isinstance(o, list):
            for v in o:
                walk(v)

    walk(m)
    return json.dumps(m).encode()


def _install_tile_patch():
    """Install the >1-sync-wait workarounds (see _split_waits_json) plus a
    tail-drain variant for the TileContext epilogue."""
    import concourse.tile as tile
    import concourse.bass_utils as bass_utils
    import concourse.bass2jax as bass2jax
    from concourse import mybir
    from concourse.vector_clock import ScopedClock

    orig_compile = bass_utils.compile_bir_kernel.__wrapped__ if hasattr(
        bass_utils.compile_bir_kernel, "__wrapped__"
    ) else bass_utils.compile_bir_kernel
    if not getattr(bass_utils, "_wait_split_installed", False):
        def patched_compile(bir_json, tmpdir, neff_name="file.neff"):
            return orig_compile(_split_waits_json(bir_json), tmpdir, neff_name)

        bass_utils.compile_bir_kernel = patched_compile
        bass2jax.compile_bir_kernel = patched_compile
        bass_utils._wait_split_installed = True

    def _patched(self, tick_clock, wait_clock):
        nc = self.nc
        carrier = nc.sync.nop(nofuse=True, hint="drain_waits")
        wait_clock.add_sem_waits(
            carrier.ins, ScopedClock({None: tick_clock.global_clock})
        )
        si = carrier.ins.sync_info
        if si is not None and len(si.on_wait) > 1:
            waits = list(si.on_wait)
            si.on_wait = waits[:1]
            carrier.ins.sync_info = si
            for w in waits[1:]:
                nop = nc.sync.nop(nofuse=True, hint="drain_waits")
                nop.ins.sync_info = mybir.SyncInfo(on_wait=[w], on_update=[])
        nc.sync.drain()

        nc.all_engine_barrier()
        assert self.sems is not None
        popped = nc._tile_sem_poison_stack.pop()
        assert popped is self._sem_poison
        nc.clear_and_free_semaphores(list(self.sems.allocated().values()))
        nc.all_engine_barrier()

    tile.TileContext._drain_and_barrier = _patched


def _ins0(ap, pos, count):
    """Insert a stride-0 (broadcast) dim into an AP at position pos."""
    import concourse.bass as bass
    dims = list(ap.ap)
    dims.insert(pos, [0, count])
    return bass.AP(tensor=ap.tensor, offset=ap.offset, ap=dims)


def _squeeze(ap):
    """Drop unit dims (the DMA balancer counts them against its 3-dim cap)."""
    import concourse.bass as bass
    dims = [list(d) for d in ap.ap if d[1] != 1]
    if not dims:
        dims = [[1, 1]]
    return bass.AP(tensor=ap.tensor, offset=ap.offset, ap=dims)


def _build_nc():
    import concourse.bass as bass
    import concourse.tile as tile
    from concourse import mybir

    f32 = mybir.dt.float32
    bf16 = mybir.dt.bfloat16
    ALU = mybir.AluOpType
    AX = mybir.AxisListType
    ACT = mybir.ActivationFunctionType

    nc = bass.Bass()
    # u pre-transposed on host to [chunk, (nn,i), (g,b)] (and cast bf16) so
    # the device DMA is contiguous; the block-diagonal stationary is built
    # on-chip by broadcasting u across the 16 nn' column slots (step-0 AP
    # dim) and multiplying by a small constant delta-mask.
    u_d = nc.dram_tensor("u3", [NCHUNK, P, G * BC], bf16, kind="ExternalInput")
    w_d = nc.dram_tensor("w", [NIN, IDIM, OD], bf16, kind="ExternalInput")
    dmask_d = nc.dram_tensor("dmask", [P, NN], bf16, kind="ExternalInput")
    maskf_d = nc.dram_tensor("mask_f", [Q, NOUT, ODIM], f32, kind="ExternalInput")
    maskb_d = nc.dram_tensor("mask_b", [Q, NOUT, ODIM], bf16, kind="ExternalInput")
    lhst0_d = nc.dram_tensor("lhst0", [P, Q], bf16, kind="ExternalInput")
    onbb_d = nc.dram_tensor("ones_bb", [Q, P], bf16, kind="ExternalInput")
    out_d = nc.dram_tensor("v_out", [BCORE, NOUT, ODIM], f32, kind="ExternalOutput")

    with tile.TileContext(nc) as tc:
        with (
            tc.tile_pool(name="singles", bufs=1) as singles,
            tc.tile_pool(name="uhat", bufs=2) as uhatp,
            tc.tile_pool(name="prod", bufs=2) as prodp,
            tc.tile_pool(name="lg", bufs=5) as lgp,
            tc.tile_pool(name="smp", bufs=2) as smp,
            tc.tile_pool(name="psu", bufs=4, space=bass.MemorySpace.PSUM) as psum_u,
            tc.tile_pool(name="psr", bufs=2, space=bass.MemorySpace.PSUM) as psum_r,
            tc.tile_pool(name="psv", bufs=1, space=bass.MemorySpace.PSUM) as psum_v,
        ):
            # ---- one-time loads ----
            w_sb = singles.tile([P, G, OD], bf16)
            nc.sync.dma_start(
                out=w_sb[:, :, :],
                in_=w_d.rearrange("(g nn) i m -> (nn i) g m", nn=NN),
            )
            dmask = singles.tile([P, NN], bf16)
            nc.sync.dma_start(out=dmask[:, :], in_=dmask_d[:, :])
            mask_f = singles.tile([Q, NOUT, ODIM], f32)
            nc.sync.dma_start(out=mask_f[:, :, :], in_=maskf_d[:, :, :])
            mask_b = singles.tile([Q, NOUT, ODIM], bf16)
            nc.sync.dma_start(out=mask_b[:, :, :], in_=maskb_d[:, :, :])
            lhst0 = singles.tile([P, Q], bf16)
            nc.sync.dma_start(out=lhst0[:, :], in_=lhst0_d[:, :])
            ones_bb = singles.tile([Q, P], bf16)
            nc.sync.dma_start(out=ones_bb[:, :], in_=onbb_d[:, :])

            # fixed buffers with persistent zero background
            cblks = []
            for ci in range(2):
                cb = singles.tile([P, G, Q], bf16, tag=f"cblk{ci}")
                nc.gpsimd.memset(cb[:, :, :], 0.0)
                cblks.append(cb)

            for c in range(NCHUNK):
                # ---- load u chunk, build block-diagonal stationary ----
                u_k = uhatp.tile([P, G, BC], bf16, tag="u_k")
                nc.sync.dma_start(
                    out=u_k[:, :, :],
                    in_=u_d[c].rearrange("p (g b) -> p g b", b=BC),
                )
                u_sb = uhatp.tile([P, G, BC, NN], bf16, tag="u_sb")
                gh = G // 2
                for h, eng in ((0, nc.vector), (1, nc.gpsimd)):
                    sl = slice(h * gh, (h + 1) * gh)
                    eng.tensor_tensor(
                        u_sb[:, sl, :, :],
                        _ins0(u_k[:, sl, :], 3, NN),
                        _ins0(_ins0(dmask[:, :], 1, gh), 2, BC),
                        op=ALU.mult,
                    )

                # ---- uhat: one matmul per g, drain 3 g's per bank ----
                uhat = uhatp.tile([P, G, OD], bf16)
                for t3 in range(G // 3):
                    ps = psum_u.tile([P, 3, OD], f32)
                    for j in range(3):
                        g = t3 * 3 + j
                        nc.tensor.matmul(
                            ps[:, j, :], u_sb[:, g, :, :], w_sb[:, g, :],
                            start=True, stop=True,
                        )
                    sl = slice(t3 * 3, (t3 + 1) * 3)
                    if t3 % 2 == 0:
                        nc.scalar.copy(uhat[:, sl, :], ps[:, :, :])
                    else:
                        nc.vector.tensor_copy(uhat[:, sl, :], ps[:, :, :])

                # ---- 3 routing iterations ----
                lg = None
                for it in range(3):
                    if it > 0:
                        # softmax over o (innermost free dim) -> scatter cblk
                        mx = smp.tile([P, G], f32, tag="mx")
                        nc.vector.tensor_reduce(mx[:, :], lg[:, :, :], axis=AX.X, op=ALU.max)
                        tsb = smp.tile([P, G, NOUT], f32, tag="tsb")
                        nc.vector.tensor_tensor(
                            tsb[:, :, :], lg[:, :, :], _ins0(mx[:, :], 2, NOUT),
                            op=ALU.subtract,
                        )
                        nc.scalar.activation(tsb[:, :, :], tsb[:, :, :], ACT.Exp)
                        sm = smp.tile([P, G], f32, tag="sm")
                        nc.vector.tensor_reduce(sm[:, :], tsb[:, :, :], axis=AX.X, op=ALU.add)
                        ri = smp.tile([P, G], f32, tag="ri")
                        nc.vector.reciprocal(ri[:, :], sm[:, :])
                        cr = smp.tile([P, G, NOUT], bf16, tag="cr")
                        nc.vector.tensor_tensor(
                            cr[:, :, :], tsb[:, :, :], _ins0(ri[:, :], 2, NOUT),
                            op=ALU.mult,
                        )
                        cb = cblks[(2 * c + it) % 2]
                        for b in range(BC):
                            nc.sync.dma_start(
                                out=cb[b * NN:(b + 1) * NN, :, b * NOUT:(b + 1) * NOUT],
                                in_=cr[b * NN:(b + 1) * NN, :, :],
                            )

                    # R1: s = sum_n c*uhat via PE, accumulated over g
                    rp = psum_r.tile([Q, OD], f32)
                    for g in range(G):
                        lh = lhst0[:, :] if it == 0 else cb[:, g, :]
                        nc.tensor.matmul(
                            rp[:, :], lh, uhat[:, g, :],
                            start=(g == 0), stop=(g == G - 1),
                        )
                    masked = smp.tile([Q, NOUT, ODIM], f32, tag="masked")
                    nc.vector.tensor_tensor(
                        masked[:, :, :],
                        rp[:, :].rearrange("q (o d) -> q o d", o=NOUT),
                        mask_f[:, :, :], op=ALU.mult,
                    )
                    s_t = smp.tile([Q, ODIM], f32, tag="s_t")
                    nc.vector.tensor_reduce(
                        s_t[:, :], masked[:, :, :].rearrange("q o d -> q d o"),
                        axis=AX.X, op=ALU.add,
                    )
                    if it == 0:
                        nc.scalar.mul(s_t[:, :], s_t[:, :], 1.0 / NOUT)

                    # squash: v = s * sqrt(n2)/(1+n2)
                    sq = smp.tile([Q, ODIM], f32, tag="sq")
                    n2 = smp.tile([Q, 1], f32, tag="n2")
                    nc.vector.tensor_tensor(sq[:, :], s_t[:, :], s_t[:, :], op=ALU.mult)
                    nc.vector.tensor_reduce(n2[:, :], sq[:, :], axis=AX.X, op=ALU.add)
                    rt = smp.tile([Q, 1], f32, tag="rt")
                    nc.scalar.sqrt(rt[:, :], n2[:, :])
                    dn = smp.tile([Q, 1], f32, tag="dn")
                    nc.vector.tensor_scalar_add(dn[:, :], n2[:, :], 1.0)
                    rv = smp.tile([Q, 1], f32, tag="rv")
                    nc.vector.reciprocal(rv[:, :], dn[:, :])
                    fc = smp.tile([Q, 1], f32, tag="fc")
                    nc.vector.tensor_tensor(fc[:, :], rt[:, :], rv[:, :], op=ALU.mult)
                    v_t = smp.tile([Q, ODIM], f32, tag="v_t")
                    nc.vector.tensor_scalar_mul(v_t[:, :], s_t[:, :], fc[:, :])

                    if it == 2:
                        nc.sync.dma_start(
                            out=out_d[c * BC:(c + 1) * BC].rearrange("b o d -> (b o) d"),
                            in_=v_t[:, :],
                        )
                        continue

                    # broadcast v to all partitions: vmask (o-diag) + PE ones
                    vm = smp.tile([Q, NOUT, ODIM], bf16, tag="vm")
                    nc.vector.tensor_tensor(
                        vm[:, :, :], mask_b[:, :, :], _ins0(v_t[:, :], 1, NOUT),
                        op=ALU.mult,
                    )
                    pv = psum_v.tile([P, OD], f32)
                    nc.tensor.matmul(
                        pv[:, :], ones_bb[:, :],
                        vm[:, :, :].rearrange("q o d -> q (o d)"),
                        start=True, stop=True,
                    )
                    vb = smp.tile([P, OD], bf16, tag="vb")
                    nc.scalar.copy(vb[:, :], pv[:, :])

                    # R2: a[p,(g,o)] = sum_d uhat * vb
                    a_t = lgp.tile([P, G, NOUT], f32, tag="a")
                    for qb in range(G // GSUB):
                        sl = slice(qb * GSUB, (qb + 1) * GSUB)
                        pr = prodp.tile([P, GSUB, NOUT, ODIM], bf16)
                        eng = nc.vector if qb % 2 == 0 else nc.gpsimd
                        eng.tensor_tensor(
                            pr[:, :, :, :],
                            uhat[:, sl, :].rearrange("p g (o d) -> p g o d", o=NOUT),
                            _ins0(vb[:, :].rearrange("p (o d) -> p o d", o=NOUT), 1, GSUB),
                            op=ALU.mult,
                        )
                        nc.vector.tensor_reduce(
                            a_t[:, sl, :], pr[:, :, :, :], axis=AX.X, op=ALU.add,
                        )
                    if it == 0:
                        lg = a_t
                    else:
                        lg2 = lgp.tile([P, G, NOUT], f32, tag="lg2")
                        nc.vector.tensor_tensor(
                            lg2[:, :, :], lg[:, :, :], a_t[:, :, :], op=ALU.add,
                        )
                        lg = lg2
    return nc


def _make_consts():
    bo = np.arange(Q) // NOUT          # b index of q
    oo = np.arange(Q) % NOUT           # o index of q
    mask = np.zeros((Q, NOUT, ODIM), np.float32)
    mask[np.arange(Q), oo, :] = 1.0
    pb = np.arange(P) // NN            # b index of p
    lhst0 = (pb[:, None] == bo[None, :]).astype(ml_dtypes.bfloat16)
    ones_bb = (bo[:, None] == pb[None, :]).astype(ml_dtypes.bfloat16)
    knn = np.arange(P) // IDIM         # nn index of K-partition
    dmask = (knn[:, None] == np.arange(NN)[None, :]).astype(ml_dtypes.bfloat16)
    return (
        dmask,
        mask,
        mask.astype(ml_dtypes.bfloat16),
        lhst0,
        ones_bb,
    )


def _make_runner(nc):
    import jax
    from jax.sharding import Mesh, PartitionSpec
    from jax.experimental.shard_map import shard_map
    from concourse import bass2jax, mybir

    bass2jax.install_neuronx_cc_hook()

    partition_name = (
        nc.partition_id_tensor.name if nc.partition_id_tensor else None
    )
    in_names, out_names, out_avals = [], [], []
    for alloc in nc.m.functions[0].allocations:
        if not isinstance(alloc, mybir.MemoryLocationSet):
            continue
        name = alloc.memorylocations[0].name
        if alloc.kind == "ExternalInput":
            if name != partition_name:
                in_names.append(name)
        elif alloc.kind == "ExternalOutput":
            out_names.append(name)
            out_avals.append(
                jax.core.ShapedArray(
                    tuple(alloc.tensor_shape), mybir.dt.np(alloc.dtype)
                )
            )
    n_params = len(in_names)
    all_names = list(in_names) + list(out_names)
    if partition_name is not None:
        all_names.append(partition_name)
    all_names = tuple(all_names)
    donate = tuple(range(n_params, n_params + len(out_names)))

    def _body(*args):
        operands = list(args)
        if partition_name is not None:
            operands.append(bass2jax.partition_id_tensor())
        outs = bass2jax._bass_exec_p.bind(
            *operands,
            out_avals=tuple(out_avals),
            in_names=all_names,
            out_names=tuple(out_names),
            lowering_input_output_aliases=(),
            sim_require_finite=False,
            sim_require_nnan=False,
            nc=nc,
        )
        return tuple(outs)

    devices = jax.devices()[:NCORES]
    mesh = Mesh(np.asarray(devices), ("core",))
    shard = {"u3"}
    in_specs = tuple(
        PartitionSpec("core") if nm in shard else PartitionSpec()
        for nm in in_names
    ) + (PartitionSpec("core"),) * len(out_names)
    out_specs = (PartitionSpec("core"),) * len(out_names)
    fn = jax.jit(
        shard_map(
            _body, mesh=mesh, in_specs=in_specs, out_specs=out_specs,
            check_rep=False,
        ),
        donate_argnums=donate,
        keep_unused=True,
    )
    return fn, in_names


def _ensure():
    if "fn" in _STATE:
        return
    _install_tile_patch()
    nc = _build_nc()
    fn, in_names = _make_runner(nc)
    consts = _make_consts()
    _STATE.update(nc=nc, fn=fn, in_names=in_names, consts=consts)
    # warm: trace + neuron compile + device transfer happen here, not on
    # the timed call
    zu = np.zeros((NCORES * NCHUNK, P, G * BC), ml_dtypes.bfloat16)
    zw = np.zeros((NIN, IDIM, OD), ml_dtypes.bfloat16)
    np.asarray(_run(zu, zw)[0])


def _run(u3, W):
    zeros = np.zeros((B, NOUT, ODIM), np.float32)
    return _STATE["fn"](u3, W, *_STATE["consts"], zeros)


def _pretranspose(u):
    # [b, (g,nn), i] -> [global_chunk, (nn,i), (g,b)], cast bf16
    return np.ascontiguousarray(
        u.reshape(NCORES, NCHUNK, BC, G, NN, IDIM)
        .transpose(0, 1, 4, 5, 3, 2)
        .reshape(NCORES * NCHUNK, P, G * BC)
        .astype(ml_dtypes.bfloat16)
    )


def kernel(u, W):
    _ensure()
    u = np.asarray(u, dtype=np.float32)
    W = np.asarray(W, dtype=np.float32).astype(ml_dtypes.bfloat16)
    (out,) = _run(_pretranspose(u), W)
    return np.asarray(out)


_ensure()
